# revision 1
# baseline (speedup 1.0000x reference)
"""Trainium2 Bass kernel for nn_Net_LSTM (B=128, T=512, H=256, H1=128).

Strategy (data-parallel over batch, 8 cores, no collectives):
  - Each core runs the full T=512 LSTM recurrence for its 16 batch rows.
  - Matmul orientation: weights stationary (lhsT = W_hh.T tiles), so the
    per-step output is gates^T in a hidden-major layout [128p, (k,b)] that
    keeps every elementwise op at full 128 partitions.
  - Input-gate contribution xg[t] = x[b,t]*w_ih + (b_ih+b_hh) is built once
    via per-partition tensor_scalar ops (bf16) and injected into PSUM each
    step with an identity matmul, so the recurrent matmuls accumulate onto
    it (start=False).
  - LSTM cell: sigmoid(f,i) in one ACT op, tanh(g) ACT, t2 = sig_f*c (DVE),
    c' = sig_i*tanh_g + t2 fused into ONE tensor_tensor_scan over
    interleaved columns, tanh(c') ACT, h = sig_o*th (DVE).
  - Gates live in 3 separate double-buffered PSUM banks (fi / g / o) so ACT
    reads never touch a bank the PE is still writing.
"""

import os

import numpy as np
import ml_dtypes

B, T, H, H1 = 128, 512, 256, 128
NCORES = 8
BS = B // NCORES  # 16 batch rows per core
NK = H // 128  # 2 hidden blocks (contraction halves)
BF16 = ml_dtypes.bfloat16

# Gate order in W_hh rows (torch): i [0:256], f [256:512], g [512:768], o [768:1024]
# Our M-tile order: (f,k0) (f,k1) (i,k0) (i,k1) (g,k0) (g,k1) (o,k0) (o,k1)
_PERM = np.concatenate(
    [np.arange(256, 512), np.arange(0, 256), np.arange(512, 768), np.arange(768, 1024)]
)

# recurrence weight dtype: "bf16" (fast) or "f32" (exact)
W_DTYPE = os.environ.get("LSTM_KERNEL_WDT", "bf16")

_PROGRAM_CACHE = {}


def _build_program(wdt_name: str, t_steps: int = T):
    from contextlib import ExitStack

    import concourse.bacc as bacc
    import concourse.mybir as mybir
    import concourse.tile as tile

    f32 = mybir.dt.float32
    bf16 = mybir.dt.bfloat16
    wdt = bf16 if wdt_name == "bf16" else f32
    AF = mybir.ActivationFunctionType
    ALU = mybir.AluOpType

    nc = bacc.Bacc("TRN2", target_bir_lowering=False, debug=False, num_devices=NCORES)

    # ---- DRAM I/O ----
    xbc_d = nc.dram_tensor("xbc", [128, T * BS], bf16, kind="ExternalInput")
    whh_d = nc.dram_tensor("whhT", [H, 4 * H], wdt, kind="ExternalInput")
    wih_d = nc.dram_tensor("wih", [128, 8], f32, kind="ExternalInput")
    bias_d = nc.dram_tensor("bias", [128, 8], f32, kind="ExternalInput")
    ident_d = nc.dram_tensor("ident", [128, 128], bf16, kind="ExternalInput")
    w1t_d = nc.dram_tensor("w1t", [H, H1], f32, kind="ExternalInput")
    b1_d = nc.dram_tensor("b1", [H1, 1], f32, kind="ExternalInput")
    w2t_d = nc.dram_tensor("w2t", [H1, 1], f32, kind="ExternalInput")
    b2_d = nc.dram_tensor("b2", [1, 1], f32, kind="ExternalInput")
    y_d = nc.dram_tensor("y", [1, BS], f32, kind="ExternalOutput")

    with tile.TileContext(nc) as tc, ExitStack() as ctx:
        sb = ctx.enter_context(tc.tile_pool(name="sb", bufs=1))
        ps = ctx.enter_context(tc.tile_pool(name="ps", bufs=2, space="PSUM"))

        # ---- persistent SBUF tiles ----
        xbc = sb.tile([128, T * BS], bf16)  # x broadcast: col = t*16+b
        xg = sb.tile([128, T, 8, BS], bf16)  # xg+bias: (t, tile m, b)
        whh = sb.tile([128, NK * 4 * H], wdt)  # (k_in, permuted gate col)
        wih = sb.tile([128, 8], f32)
        bias = sb.tile([128, 8], f32)
        ident = sb.tile([128, 128], bf16)
        sigfi = sb.tile([128, 128], f32)  # [f-region 64 | i-region 64], stride-2
        tgb = sb.tile([128, 64], f32)  # evens 0, odds tanh(g)
        cbuf = sb.tile([128, 64], f32)  # odds = c state
        thb = sb.tile([128, 32], f32)
        sigo = sb.tile([128, 32], f32)
        hbuf = sb.tile([128, 2 * BS], wdt)  # h state: col = k*16+b
        hf32 = sb.tile([128, 2 * BS], f32)
        w1t = sb.tile([128, NK * H1], f32)
        b1s = sb.tile([128, 1], f32)
        w2t = sb.tile([128, 1], f32)
        b2s = sb.tile([1, 1], f32)
        asb = sb.tile([128, BS], f32)
        ysb = sb.tile([1, BS], f32)

        # ---- prologue: loads ----
        nc.sync.dma_start(xbc[:], xbc_d[:])
        for k in range(NK):
            nc.sync.dma_start(
                whh[:, k * 4 * H : (k + 1) * 4 * H], whh_d[k * 128 : (k + 1) * 128, :]
            )
            nc.sync.dma_start(
                w1t[:, k * H1 : (k + 1) * H1], w1t_d[k * 128 : (k + 1) * 128, :]
            )
        nc.sync.dma_start(wih[:], wih_d[:])
        nc.sync.dma_start(bias[:], bias_d[:])
        nc.sync.dma_start(ident[:], ident_d[:])
        nc.sync.dma_start(b1s[:], b1_d[:])
        nc.sync.dma_start(w2t[:], w2t_d[:])
        nc.sync.dma_start(b2s[:], b2_d[:])

        nc.vector.memset(tgb[:], 0.0)
        nc.vector.memset(cbuf[:], 0.0)
        nc.vector.memset(hbuf[:], 0.0)
        nc.vector.memset(sigo[:], 0.0)
        # warm the sigmoid/tanh ACT table set early (overlaps DMAs)
        nc.scalar.activation(thb[:, 0:1], sigo[:, 0:1], AF.Sigmoid)

        # ---- xg = x*w_ih + bias, per M-tile (per-partition scalars) ----
        xbc_v = xbc[:].rearrange("p (t b) -> p t b", b=BS)
        for m in range(8):
            nc.vector.tensor_scalar(
                xg[:, :, m, :],
                xbc_v,
                wih[:, m : m + 1],
                bias[:, m : m + 1],
                op0=ALU.mult,
                op1=ALU.add,
            )

        # strided views for the cell math
        sig_f = sigfi[:, 0:64:2]  # sigma(f)
        t2_out = sigfi[:, 65:128:2]  # t2 = sig_f * c (odds of i-region)
        d1_scan = sigfi[:, 64:128]  # [sig_i | t2] interleaved
        sig_all_out = sigfi[:, 0:128:2]  # ACT1 target (f then i evens)
        tg_odd = tgb[:, 1:64:2]
        c_odd = cbuf[:, 1:64:2]

        xg_t = xg[:].rearrange("p t m b -> p t (m b)")

        # ---- recurrence ----
        for t in range(t_steps):
            fi = ps.tile([128, 64], f32, tag="fi")
            gb = ps.tile([128, 32], f32, tag="g")
            ob = ps.tile([128, 32], f32, tag="o")

            # inject xg (+bias) into the three gate banks
            nc.tensor.matmul(fi[:], ident[:], xg_t[:, t, 0:64], start=True, stop=False)
            nc.tensor.matmul(gb[:], ident[:], xg_t[:, t, 64:96], start=True, stop=False)
            nc.tensor.matmul(ob[:], ident[:], xg_t[:, t, 96:128], start=True, stop=False)

            # recurrent matmuls: 8 M-tiles x 2 contraction halves
            for tp in range(8):
                if tp < 4:
                    dst = fi[:, tp * 16 : (tp + 1) * 16]
                elif tp < 6:
                    dst = gb[:, (tp - 4) * 16 : (tp - 3) * 16]
                else:
                    dst = ob[:, (tp - 6) * 16 : (tp - 5) * 16]
                for ki in range(NK):
                    nc.tensor.matmul(
                        dst,
                        whh[:, ki * 4 * H + tp * 128 : ki * 4 * H + (tp + 1) * 128],
                        hbuf[:, ki * BS : (ki + 1) * BS],
                        start=False,
                        stop=(tp in (3, 5, 7) and ki == NK - 1),
                    )

            # cell math
            nc.scalar.activation(sig_all_out, fi[:], AF.Sigmoid)  # sig(f), sig(i)
            nc.scalar.activation(tg_odd, gb[:], AF.Tanh)  # tanh(g)
            nc.vector.tensor_tensor(t2_out, sig_f, c_odd, ALU.mult)  # t2 = sig_f*c
            nc.vector.tensor_tensor_scan(  # c' = sig_i*tg + t2
                cbuf[:], tgb[:], d1_scan, 0.0, op0=ALU.mult, op1=ALU.add
            )
            nc.scalar.activation(sigo[:], ob[:], AF.Sigmoid)  # sig(o)
            nc.scalar.activation(thb[:], c_odd, AF.Tanh)  # tanh(c')
            nc.vector.tensor_tensor(hbuf[:], sigo[:], thb[:], ALU.mult)
            if t == t_steps - 1:
                nc.vector.tensor_tensor(hf32[:], sigo[:], thb[:], ALU.mult)

        # ---- head: y = relu(h @ W1.T + b1) @ W2.T + b2 ----
        zps = ps.tile([128, BS], f32, tag="z", bufs=1)
        for ki in range(NK):
            nc.tensor.matmul(
                zps[:],
                w1t[:, ki * H1 : (ki + 1) * H1],
                hf32[:, ki * BS : (ki + 1) * BS],
                start=(ki == 0),
                stop=(ki == NK - 1),
            )
        nc.scalar.activation(asb[:], zps[:], AF.Relu, bias=b1s[:, 0:1])
        yps = ps.tile([1, BS], f32, tag="y", bufs=1)
        nc.tensor.matmul(yps[:], w2t[:], asb[:], start=True, stop=True)
        nc.vector.tensor_scalar(ysb[:], yps[:], b2s[:, 0:1], None, op0=ALU.add)
        nc.sync.dma_start(y_d[:], ysb[:])

    nc.compile()
    return nc


def _get_program(wdt_name: str):
    key = wdt_name
    if key not in _PROGRAM_CACHE:
        _PROGRAM_CACHE[key] = _build_program(wdt_name)
    return _PROGRAM_CACHE[key]


def _make_in_maps(x, W_ih, W_hh, b_ih, b_hh, W1, b1, W2, b2, wdt_name):
    np_wdt = BF16 if wdt_name == "bf16" else np.float32
    whhT = np.ascontiguousarray(W_hh.T[:, _PERM]).astype(np_wdt)  # [256, 1024]
    wih_p = W_ih[:, 0][_PERM].astype(np.float32)  # [1024]
    bias_p = (b_ih + b_hh)[_PERM].astype(np.float32)
    wih_cols = np.ascontiguousarray(wih_p.reshape(8, 128).T)  # [128, 8]
    bias_cols = np.ascontiguousarray(bias_p.reshape(8, 128).T)
    ident = np.eye(128, dtype=BF16)
    w1t = np.ascontiguousarray(W1.T).astype(np.float32)  # [256, 128]
    b1c = b1.reshape(H1, 1).astype(np.float32)
    w2t = np.ascontiguousarray(W2.T).astype(np.float32)  # [128, 1]
    b2c = b2.reshape(1, 1).astype(np.float32)

    in_maps = []
    for c in range(NCORES):
        xs = x[c * BS : (c + 1) * BS, :, 0]  # [16, 512]
        xT = np.ascontiguousarray(xs.T).reshape(1, T * BS).astype(BF16)
        xbc = np.ascontiguousarray(np.broadcast_to(xT, (128, T * BS)))
        in_maps.append(
            {
                "xbc": xbc,
                "whhT": whhT,
                "wih": wih_cols,
                "bias": bias_cols,
                "ident": ident,
                "w1t": w1t,
                "b1": b1c,
                "w2t": w2t,
                "b2": b2c,
            }
        )
    return in_maps


LAST_RESULTS = None  # stash for test.py (exec time / profile)


def kernel(x, W_ih, W_hh, b_ih, b_hh, W1, b1, W2, b2, trace=False, **trace_kw):
    global LAST_RESULTS
    from concourse import bass_utils

    wdt_name = W_DTYPE
    nc = _get_program(wdt_name)
    in_maps = _make_in_maps(
        np.asarray(x, np.float32),
        np.asarray(W_ih, np.float32),
        np.asarray(W_hh, np.float32),
        np.asarray(b_ih, np.float32),
        np.asarray(b_hh, np.float32),
        np.asarray(W1, np.float32),
        np.asarray(b1, np.float32),
        np.asarray(W2, np.float32),
        np.asarray(b2, np.float32),
        wdt_name,
    )
    res = bass_utils.run_bass_kernel_spmd(
        nc, in_maps, core_ids=list(range(NCORES)), trace=trace, **trace_kw
    )
    LAST_RESULTS = res
    y = np.concatenate(
        [np.asarray(r["y"], np.float32).reshape(BS, 1) for r in res.results], axis=0
    )
    return y


# revision 9
# speedup vs baseline: 2381.2599x; 2381.2599x over previous
"""Trainium2 Bass kernel for nn_Net_LSTM (B=128, T=512, H=256, H1=128).

Strategy (data-parallel over batch, 8 cores, no collectives):
  - Each core runs the full T=512 LSTM recurrence for its 16 batch rows.
  - Matmul orientation: weights stationary (lhsT = W_hh.T tiles), so the
    per-step output is gates^T in a hidden-major layout [128p, (k,b)] that
    keeps every elementwise op at full 128 partitions.
  - Input-gate contribution xg[t] = x[b,t]*w_ih + (b_ih+b_hh) is built once
    via per-partition tensor_scalar ops (bf16) and injected into PSUM each
    step with an identity matmul, so the recurrent matmuls accumulate onto
    it (start=False).
  - LSTM cell: sigmoid(f,i) in one ACT op, tanh(g) ACT, t2 = sig_f*c (DVE),
    c' = sig_i*tanh_g + t2 fused into ONE tensor_tensor_scan over
    interleaved columns, tanh(c') ACT, h = sig_o*th (DVE).
  - Gates live in 3 separate double-buffered PSUM banks (fi / g / o) so ACT
    reads never touch a bank the PE is still writing.
"""

import os

import numpy as np
import ml_dtypes

B, T, H, H1 = 128, 512, 256, 128
NCORES = 8
BS = B // NCORES  # 16 batch rows per core
NK = H // 128  # 2 hidden blocks (contraction halves)
BF16 = ml_dtypes.bfloat16

# Gate order in W_hh rows (torch): i [0:256], f [256:512], g [512:768], o [768:1024]
# Our M-tile order: (f,k0) (f,k1) (i,k0) (i,k1) (g,k0) (g,k1) (o,k0) (o,k1)
_PERM = np.concatenate(
    [np.arange(256, 512), np.arange(0, 256), np.arange(512, 768), np.arange(768, 1024)]
)

# recurrence weight dtype: "bf16" (fast) or "f32" (exact)
W_DTYPE = os.environ.get("LSTM_KERNEL_WDT", "bf16")

_PROGRAM_CACHE = {}


def _build_program(wdt_name: str, t_steps: int = T, repeats: int = 1):
    from contextlib import ExitStack

    import concourse.bacc as bacc
    import concourse.mybir as mybir
    import concourse.tile as tile

    f32 = mybir.dt.float32
    bf16 = mybir.dt.bfloat16
    wdt = bf16 if wdt_name == "bf16" else f32
    AF = mybir.ActivationFunctionType
    ALU = mybir.AluOpType

    nc = bacc.Bacc("TRN2", target_bir_lowering=False, debug=False, num_devices=NCORES)

    # ---- DRAM I/O ----
    xbc_d = nc.dram_tensor("xbc", [128, T * BS], bf16, kind="ExternalInput")
    whh_d = nc.dram_tensor("whhT", [H, 4 * H], wdt, kind="ExternalInput")
    wih_d = nc.dram_tensor("wih", [128, 8], f32, kind="ExternalInput")
    bias_d = nc.dram_tensor("bias", [128, 8], f32, kind="ExternalInput")
    ident_d = nc.dram_tensor("ident", [128, 128], bf16, kind="ExternalInput")
    w1t_d = nc.dram_tensor("w1t", [H, H1], f32, kind="ExternalInput")
    b1_d = nc.dram_tensor("b1", [H1, 1], f32, kind="ExternalInput")
    w2t_d = nc.dram_tensor("w2t", [H1, 1], f32, kind="ExternalInput")
    b2_d = nc.dram_tensor("b2", [1, 1], f32, kind="ExternalInput")
    y_d = nc.dram_tensor("y", [1, BS], f32, kind="ExternalOutput")

    with tile.TileContext(nc) as tc, ExitStack() as ctx:
        sb = ctx.enter_context(tc.tile_pool(name="sb", bufs=1))
        ps = ctx.enter_context(tc.tile_pool(name="ps", bufs=2, space="PSUM"))

        # ---- persistent SBUF tiles ----
        xbc = sb.tile([128, T * BS], bf16)  # x broadcast: col = t*16+b
        xg = sb.tile([128, T, 8, BS], bf16)  # xg+bias: (t, tile m, b)
        whh = sb.tile([128, NK * 4 * H], wdt)  # (k_in, permuted gate col)
        wih = sb.tile([128, 8], f32)
        bias = sb.tile([128, 8], f32)
        ident = sb.tile([128, 128], bf16)
        sigfi = sb.tile([128, 128], f32)  # [f-region 64 | i-region 64], stride-2
        tgb = sb.tile([128, 64], f32)  # evens 0, odds tanh(g)
        thb = sb.tile([128, 32], f32)
        sigo = sb.tile([128, 32], f32)
        hbuf = sb.tile([128, 2 * BS], wdt)  # h state: col = k*16+b
        hf32 = sb.tile([128, 2 * BS], f32)
        w1t = sb.tile([128, NK * H1], f32)
        b1s = sb.tile([128, 1], f32)
        w2t = sb.tile([128, 1], f32)
        b2s = sb.tile([1, 1], f32)
        asb = sb.tile([128, BS], f32)
        ysb = sb.tile([1, BS], f32)

        # ---- prologue: loads ----
        nc.sync.dma_start(xbc[:], xbc_d[:])
        for k in range(NK):
            nc.sync.dma_start(
                whh[:, k * 4 * H : (k + 1) * 4 * H], whh_d[k * 128 : (k + 1) * 128, :]
            )
            nc.sync.dma_start(
                w1t[:, k * H1 : (k + 1) * H1], w1t_d[k * 128 : (k + 1) * 128, :]
            )
        nc.sync.dma_start(wih[:], wih_d[:])
        nc.sync.dma_start(bias[:], bias_d[:])
        nc.sync.dma_start(ident[:], ident_d[:])
        nc.sync.dma_start(b1s[:], b1_d[:])
        nc.sync.dma_start(w2t[:], w2t_d[:])
        nc.sync.dma_start(b2s[:], b2_d[:])

        cbuf = sb.tile([128, 64], f32)  # odds = c state
        c_odd = cbuf[:, 1:64:2]

        nc.vector.memset(tgb[:], 0.0)
        nc.vector.memset(cbuf[:], 0.0)
        nc.vector.memset(hbuf[:], 0.0)
        nc.vector.memset(sigo[:], 0.0)
        # warm the sigmoid/tanh ACT table set early (overlaps DMAs)
        nc.scalar.activation(thb[:, 0:1], sigo[:, 0:1], AF.Sigmoid)

        # ---- xg = x*w_ih + bias, per M-tile (per-partition scalars) ----
        xbc_v = xbc[:].rearrange("p (t b) -> p t b", b=BS)
        for m in range(8):
            nc.vector.tensor_scalar(
                xg[:, :, m, :],
                xbc_v,
                wih[:, m : m + 1],
                bias[:, m : m + 1],
                op0=ALU.mult,
                op1=ALU.add,
            )

        # strided views for the cell math
        sig_f = sigfi[:, 0:64:2]  # sigma(f)
        t2_out = sigfi[:, 65:128:2]  # t2 = sig_f * c (odds of i-region)
        d1_scan = sigfi[:, 64:128]  # [sig_i | t2] interleaved
        sig_all_out = sigfi[:, 0:128:2]  # ACT1 target (f then i evens)
        tg_odd = tgb[:, 1:64:2]

        xg_t = xg[:].rearrange("p t m b -> p t (m b)")

        def w_mm(tp, ki, dst):
            nc.tensor.matmul(
                dst,
                whh[:, ki * 4 * H + tp * 128 : ki * 4 * H + (tp + 1) * 128],
                hbuf[:, ki * BS : (ki + 1) * BS],
                start=False,
                stop=(tp in (3, 5, 7) and ki == NK - 1),
            )

        # ---- recurrence ----
        for rep in range(repeats):
            if rep > 0:  # timing-only variants re-run the recurrence
                nc.vector.memset(cbuf[:], 0.0)
                nc.vector.memset(hbuf[:], 0.0)
            for t in range(t_steps):
                fi = ps.tile([128, 64], f32, tag="fi")
                gb = ps.tile([128, 32], f32, tag="g")
                ob = ps.tile([128, 32], f32, tag="o")

                # inject xg (+bias) into the three gate banks
                nc.tensor.matmul(
                    fi[:], ident[:], xg_t[:, t, 0:64], start=True, stop=False
                )
                nc.tensor.matmul(
                    gb[:], ident[:], xg_t[:, t, 64:96], start=True, stop=False
                )
                nc.tensor.matmul(
                    ob[:], ident[:], xg_t[:, t, 96:128], start=True, stop=False
                )

                # fi-bank matmuls first, then its ACT immediately (sem placement)
                for tp in range(4):
                    for ki in range(NK):
                        w_mm(tp, ki, fi[:, tp * 16 : (tp + 1) * 16])
                nc.scalar.activation(sig_all_out, fi[:], AF.Sigmoid)  # sig(f), sig(i)
                nc.vector.tensor_tensor(t2_out, sig_f, c_odd, ALU.mult)  # sig_f*c
                for tp in range(4, 6):
                    for ki in range(NK):
                        w_mm(tp, ki, gb[:, (tp - 4) * 16 : (tp - 3) * 16])
                nc.scalar.activation(tg_odd, gb[:], AF.Tanh)  # tanh(g)
                nc.vector.tensor_tensor_scan(  # c' = sig_i*tg + t2
                    cbuf[:], tgb[:], d1_scan, 0.0, op0=ALU.mult, op1=ALU.add
                )
                for tp in range(6, 8):
                    for ki in range(NK):
                        w_mm(tp, ki, ob[:, (tp - 6) * 16 : (tp - 5) * 16])
                nc.scalar.activation(sigo[:], ob[:], AF.Sigmoid)  # sig(o)
                nc.scalar.activation(thb[:], c_odd, AF.Tanh)  # tanh(c')
                nc.vector.tensor_tensor(hbuf[:], sigo[:], thb[:], ALU.mult)
                if t == t_steps - 1 and rep == repeats - 1:
                    nc.vector.tensor_tensor(hf32[:], sigo[:], thb[:], ALU.mult)

        # ---- head: y = relu(h @ W1.T + b1) @ W2.T + b2 ----
        zps = ps.tile([128, BS], f32, tag="fi")  # reuse a recurrence bank
        for ki in range(NK):
            nc.tensor.matmul(
                zps[:],
                w1t[:, ki * H1 : (ki + 1) * H1],
                hf32[:, ki * BS : (ki + 1) * BS],
                start=(ki == 0),
                stop=(ki == NK - 1),
            )
        nc.scalar.activation(asb[:], zps[:], AF.Relu, bias=b1s[:, 0:1])
        yps = ps.tile([1, BS], f32, tag="g")  # reuse a recurrence bank
        nc.tensor.matmul(yps[:], w2t[:], asb[:], start=True, stop=True)
        nc.vector.tensor_scalar(ysb[:], yps[:], b2s[:, 0:1], None, op0=ALU.add)
        nc.sync.dma_start(y_d[:], ysb[:])

    nc.compile()
    return nc


def _get_program(wdt_name: str):
    key = wdt_name
    if key not in _PROGRAM_CACHE:
        _PROGRAM_CACHE[key] = _build_program(wdt_name)
    return _PROGRAM_CACHE[key]


def _make_in_maps(x, W_ih, W_hh, b_ih, b_hh, W1, b1, W2, b2, wdt_name):
    np_wdt = BF16 if wdt_name == "bf16" else np.float32
    whhT = np.ascontiguousarray(W_hh.T[:, _PERM]).astype(np_wdt)  # [256, 1024]
    wih_p = W_ih[:, 0][_PERM].astype(np.float32)  # [1024]
    bias_p = (b_ih + b_hh)[_PERM].astype(np.float32)
    wih_cols = np.ascontiguousarray(wih_p.reshape(8, 128).T)  # [128, 8]
    bias_cols = np.ascontiguousarray(bias_p.reshape(8, 128).T)
    ident = np.eye(128, dtype=BF16)
    w1t = np.ascontiguousarray(W1.T).astype(np.float32)  # [256, 128]
    b1c = b1.reshape(H1, 1).astype(np.float32)
    w2t = np.ascontiguousarray(W2.T).astype(np.float32)  # [128, 1]
    b2c = b2.reshape(1, 1).astype(np.float32)

    in_maps = []
    for c in range(NCORES):
        xs = x[c * BS : (c + 1) * BS, :, 0]  # [16, 512]
        xT = np.ascontiguousarray(xs.T).reshape(1, T * BS).astype(BF16)
        xbc = np.ascontiguousarray(np.broadcast_to(xT, (128, T * BS)))
        in_maps.append(
            {
                "xbc": xbc,
                "whhT": whhT,
                "wih": wih_cols,
                "bias": bias_cols,
                "ident": ident,
                "w1t": w1t,
                "b1": b1c,
                "w2t": w2t,
                "b2": b2c,
            }
        )
    return in_maps


LAST_RESULTS = None  # stash for test.py (exec time / profile)


def kernel(x, W_ih, W_hh, b_ih, b_hh, W1, b1, W2, b2, trace=False, **trace_kw):
    global LAST_RESULTS
    from concourse import bass_utils

    wdt_name = W_DTYPE
    nc = _get_program(wdt_name)
    in_maps = _make_in_maps(
        np.asarray(x, np.float32),
        np.asarray(W_ih, np.float32),
        np.asarray(W_hh, np.float32),
        np.asarray(b_ih, np.float32),
        np.asarray(b_hh, np.float32),
        np.asarray(W1, np.float32),
        np.asarray(b1, np.float32),
        np.asarray(W2, np.float32),
        np.asarray(b2, np.float32),
        wdt_name,
    )
    res = bass_utils.run_bass_kernel_spmd(
        nc, in_maps, core_ids=list(range(NCORES)), trace=trace, **trace_kw
    )
    LAST_RESULTS = res
    y = np.concatenate(
        [np.asarray(r["y"], np.float32).reshape(BS, 1) for r in res.results], axis=0
    )
    return y
